# revision 25
# baseline (speedup 1.0000x reference)
"""AWQ W4A8 linear (x:[8,32,8192] f32, qweight:[8192,8192] int4-range int32,
w_scales/bias:[8192] f32) -> [8,32,8192] f32 on 8 trn2 NeuronCores.

Column-parallel sharding: qweight / w_scales / bias are split along N
(output channels) across the 8 cores; x — quantized per-token on the host
exactly as the reference does — and the per-token act_scales are
replicated. Each core computes an exact integer GEMM of
x_q [256,8192] @ qw_shard [8192,1024], applies the per-token/per-channel
dequant + bias epilogue, and writes its [256,1024] slice; the host
concatenates the slices.

Numerics: x_q in [-127,127] ships as bf16 and qw in [-8,7] ships as fp8e4
(both exactly representable), and the PE's mixed bf16 x fp8 matmul
accumulates exactly in fp32 PSUM (every product/sum is an integer < 2^24),
so the result matches the reference bit-for-bit while weight HBM traffic
drops 4x vs the int32 input encoding.

The device program is raw Bass (no TileContext) with hand-placed
semaphores. The PE stream is the hard floor (256 matmuls of N=512 at
~213 ns warm = ~55 us), so the design keeps the PE saturated:

- Weights (8 MB) and activations (4 MB) are fully resident in SBUF — no
  slot ring, so DMA runs arbitrarily far ahead with zero backpressure.
- Early weight ranges alternate between the two hardware DGE queues
  (sync + scalar) so the first ~1 MB lands at twice the single-queue
  ramp rate; x rides the gpsimd SWDGE queue (piece 0 goes early on
  scalar), with the 1 MB of scale/bias constants last.
- A short burst of dummy matmuls on garbage SBUF runs during the DGE
  spin-up so the HAM clock-gate reaches 2.4 GHz in dead time. (Dummy
  work must stay minimal: sustained extra PE load trips the P0 power
  downclock, 2.4 -> 2.0 GHz, slowing every real matmul.)
- Per-range completion semaphores: a DMA's +16 lands as 16 per-spray-
  engine increments which interleave across in-flight DMAs, so a single
  counting semaphore is NOT range-ordered.
- Chunks 0..47 interleave all 4 PSUM tiles; chunks 48..63 run
  tile-by-tile so each tile's dequant epilogue (DVE) and output store
  (sync queue) overlap the tail matmuls; the last tile's epilogue is
  further split in halves. Only ~2 us of tail is exposed.
"""

from contextlib import ExitStack

import numpy as np

import concourse.bass as bass
import concourse.mybir as mybir
import concourse.bass_utils as bass_utils
from concourse.dt import dt as cdt

N_CORES = 8
P = 128
B, S, K, N = 8, 32, 8192, 8192
TOK = B * S                      # 256 tokens
NL = N // N_CORES                # 1024 output channels per core
KC = K // P                      # 64 contraction chunks of 128
EPS = 1e-8

# Weight chunk ranges, alternating sync/scalar queues (small leading
# ranges so chunk 0/1 land ASAP on both spinning-up queues).
W_SIZES = [1, 1, 2, 2, 4, 4, 8, 8, 16, 18]
X_SIZES = [2, 6, 8, 16, 32]      # x pieces: p0 on scalar, rest on gpsimd
PH1 = 48                         # chunks 0..PH1-1 interleave all 4 tiles
NWARM = 6                        # warm-up dummy matmuls (N=512)

assert sum(W_SIZES) == KC and sum(X_SIZES) == KC

_cached = None


def _ranges(sizes):
    out, acc = [], 0
    for s in sizes:
        out.append((acc, acc + s))
        acc += s
    return out


W_RANGES = _ranges(W_SIZES)
X_RANGES = _ranges(X_SIZES)


def _range_of(c, ranges):
    for i, (a, b) in enumerate(ranges):
        if a <= c < b:
            return i
    raise ValueError(c)


def _build_nc():
    nc = bass.Bass(
        "TRN2",
        target_bir_lowering=False,
        debug=False,
        enable_asserts=False,
        num_devices=N_CORES,
    )
    dt = mybir.dt

    xq_d = nc.dram_tensor("xq", [P, KC, TOK], dt.bfloat16, kind="ExternalInput")
    qw_d = nc.dram_tensor("qw", [P, KC, NL], dt.float8e4, kind="ExternalInput")
    ws_d = nc.dram_tensor("ws", [P, NL], dt.float32, kind="ExternalInput")
    bs_d = nc.dram_tensor("bs", [P, NL], dt.float32, kind="ExternalInput")
    as_d = nc.dram_tensor("asc", [P, 2], dt.float32, kind="ExternalInput")
    out_d = nc.dram_tensor("out", [2, P, NL], dt.float32, kind="ExternalOutput")

    ctx = ExitStack()
    xq_s = ctx.enter_context(nc.sbuf_tensor("xq_s", [P, KC, TOK], dt.bfloat16))
    w_s = ctx.enter_context(nc.sbuf_tensor("w_s", [P, KC, NL], dt.float8e4))
    ws_s = ctx.enter_context(nc.sbuf_tensor("ws_s", [P, NL], dt.float32))
    bs_s = ctx.enter_context(nc.sbuf_tensor("bs_s", [P, NL], dt.float32))
    as_s = ctx.enter_context(nc.sbuf_tensor("as_s", [P, 2], dt.float32))
    t_s = ctx.enter_context(nc.sbuf_tensor("t_s", [P, 4, 512], dt.float32))
    o_s = ctx.enter_context(nc.sbuf_tensor("o_s", [P, 4, 512], dt.float32))
    # never DMA'd: garbage operands for the PE warm-up burst
    dum_s = ctx.enter_context(nc.sbuf_tensor("dum_s", [P, 512], dt.bfloat16))

    ps = [
        ctx.enter_context(nc.psum_tensor(f"ps{i}", [P, 512], dt.float32))
        for i in range(4)  # (m,n): 00,01,10,11
    ]
    ps_warm = ctx.enter_context(nc.psum_tensor("ps_warm", [P, 512], dt.float32))

    sems = {}

    def sem(name):
        sems[name] = ctx.enter_context(nc.semaphore(name))
        return sems[name]

    s_wr = [sem(f"s_wr{g}") for g in range(len(W_RANGES))]
    s_xp = [sem(f"s_xp{i}") for i in range(len(X_RANGES))]
    s_cst = sem("s_cst")  # asc + ws + bs, +16 each (total 48 = all done)
    s_ps = sem("s_ps")    # tile accumulation complete, +1 per tile in order
    s_dve = sem("s_dve")  # DVE same-engine RAW ordering
    s_ep = sem("s_ep")    # epilogue complete, +1 per unit in order
    s_out = sem("s_out")  # output stores (DGE requires sync info)

    # Epilogue/store units: (m, col0, col1) within the core's NL columns.
    # The last tile's epilogue is split in halves so only ~half a tile's
    # epilogue + store is exposed after the final matmul.
    EP_UNITS = [(0, 0, 512), (0, 512, 1024), (1, 0, 512),
                (1, 512, 768), (1, 768, 1024)]
    EP_OSL = [(0, slice(0, 512)), (1, slice(0, 512)), (2, slice(0, 512)),
              (3, slice(0, 256)), (3, slice(256, 512))]
    EP_WAIT = [1, 2, 3, 4, 4]  # s_ps value each unit waits for

    def wdma(eng, r):
        a, b = W_RANGES[r]
        eng.dma_start(w_s[:, a:b, :], qw_d.ap()[:, a:b, :]).then_inc(s_wr[r], 16)

    def xdma(eng, i):
        a, b = X_RANGES[i]
        eng.dma_start(xq_s[:, a:b, :], xq_d.ap()[:, a:b, :]).then_inc(s_xp[i], 16)

    # Issue the critical first DMAs before anything else: the DGE spin-up
    # takes ~2-3us and runs while the program clears semaphores / warms up.
    # Prior executions fully drained (Block exit drains), so clearing the
    # semaphores below without a dma_reset is safe.
    wdma(nc.sync, 0)
    xdma(nc.scalar, 0)

    # Zero our semaphores up front (a previous execution of this NEFF leaves
    # them at their final values), then barrier so no engine runs ahead.
    nums = sorted(s.num for s in sems.values())
    lo = 0
    while lo < len(nums):
        hi = lo
        while hi + 1 < len(nums) and nums[hi + 1] == nums[hi] + 1:
            hi += 1
        nc.gpsimd.sem_clear(range(nums[lo], nums[hi] + 1))
        lo = hi + 1
    nc.all_engine_barrier()

    with nc.Block() as block:

        @block.sync
        def _(sync):
            for r in range(2, len(W_RANGES), 2):
                wdma(sync, r)
            for idx in range(len(EP_UNITS)):
                m, c0, c1 = EP_UNITS[idx]
                sync.wait_ge(s_ep, idx + 1)
                sync.dma_start(
                    out_d.ap()[m][:, c0:c1], o_s[:, EP_OSL[idx][0], EP_OSL[idx][1]]
                ).then_inc(s_out, 16)

        @block.scalar
        def _(scalar):
            for r in range(1, len(W_RANGES), 2):
                wdma(scalar, r)

        @block.gpsimd
        def _(gpsimd):
            for i in range(1, len(X_RANGES)):
                xdma(gpsimd, i)
            gpsimd.dma_start(as_s[:], as_d.ap()).then_inc(s_cst, 16)
            gpsimd.dma_start(ws_s[:], ws_d.ap()).then_inc(s_cst, 16)
            gpsimd.dma_start(bs_s[:], bs_d.ap()).then_inc(s_cst, 16)

        @block.tensor
        def _(tensor):
            # Warm-up burst: garbage matmuls into a scratch PSUM bank keep
            # the PE busy through the HAM activity window while the first
            # real chunks stream in.
            for _ in range(NWARM):
                tensor.matmul(
                    ps_warm.ap(), dum_s[:, 0:P], dum_s[:], start=True, stop=True
                )

            cur_wr = -1
            cur_xp = -1

            def need(c):
                nonlocal cur_wr, cur_xp
                r = _range_of(c, W_RANGES)
                if r > cur_wr:
                    tensor.wait_ge(s_wr[r], 16)
                    cur_wr = r
                i = _range_of(c, X_RANGES)
                if i > cur_xp:
                    tensor.wait_ge(s_xp[i], 16)
                    cur_xp = i

            def mm(c, m, n0, n1, stop=False, inc=False):
                inst = tensor.matmul(
                    ps[2 * m + (n0 >= 512)].ap()[:, n0 % 512 : n0 % 512 + (n1 - n0)],
                    xq_s[:, c, P * m : P * (m + 1)],
                    w_s[:, c, n0:n1],
                    start=(c == 0),
                    stop=stop,
                )
                if inc:
                    inst.then_inc(s_ps, 1)

            # Phase 1: chunks 0..PH1-1, all 4 tiles per chunk (follows DMA)
            for c in range(PH1):
                need(c)
                for m in range(2):
                    mm(c, m, 0, 512)
                    mm(c, m, 512, 1024)

            # Phase 2: chunks PH1..KC-1 tile-by-tile; each tile's epilogue
            # and store overlap the next tile's matmuls.
            need(KC - 1)
            for m, n in [(0, 0), (0, 1), (1, 0), (1, 1)]:
                for c in range(PH1, KC):
                    last = c == KC - 1
                    mm(c, m, 512 * n, 512 * (n + 1), stop=last, inc=last)

        @block.vector
        def _(vector):
            vector.wait_ge(s_cst, 48)
            for idx in range(len(EP_UNITS)):
                m, c0, c1 = EP_UNITS[idx]
                osl = EP_OSL[idx]
                bank = 2 * m + (c0 >= 512)
                psl = slice(c0 % 512, c0 % 512 + (c1 - c0))
                vector.wait_ge(s_ps, EP_WAIT[idx])
                vector.scalar_tensor_tensor(
                    t_s[:, osl[0], osl[1]],
                    ps[bank].ap()[:, psl],
                    as_s[:, m : m + 1],
                    ws_s[:, c0:c1],
                    mybir.AluOpType.mult,
                    mybir.AluOpType.mult,
                ).then_inc(s_dve, 1)
                # DVE is deeply pipelined: same-engine RAW needs a sem
                vector.wait_ge(s_dve, idx + 1)
                vector.tensor_add(
                    o_s[:, osl[0], osl[1]], t_s[:, osl[0], osl[1]], bs_s[:, c0:c1]
                ).then_inc(s_ep, 1)

    return nc, ctx


def _prep_inputs(x, qweight, w_scales, bias):
    bf16 = cdt.np(mybir.dt.bfloat16)
    fp8 = cdt.np(mybir.dt.float8e4)

    x2 = np.asarray(x, dtype=np.float32).reshape(TOK, K)
    max_abs = np.max(np.abs(x2), axis=-1, keepdims=True)
    act_scales = np.maximum(max_abs / np.float32(127.0), np.float32(EPS)).astype(
        np.float32
    )
    x_q = np.clip(np.round(x2 / act_scales), -127, 127).astype(np.float32)

    # [TOK, K] -> K-major [P, KC, TOK]: xq[p, c, t] = x_q[t, c*128 + p]
    xq = np.ascontiguousarray(
        x_q.T.reshape(KC, P, TOK).transpose(1, 0, 2).astype(bf16)
    )

    # act_scales arranged per m-tile: asc[p, m] = act_scales[m*128 + p]
    asc = np.ascontiguousarray(act_scales.reshape(2, P).T.astype(np.float32))

    # int4-range weights are exactly representable in fp8 e4m3
    qw8 = np.asarray(qweight, dtype=np.int8).astype(fp8)
    w_scales = np.asarray(w_scales, dtype=np.float32)
    bias = np.asarray(bias, dtype=np.float32)

    in_maps = []
    for i in range(N_CORES):
        sl = slice(i * NL, (i + 1) * NL)
        # [K, NL] -> p-major [P, KC, NL]: qw[p, c, n] = shard[c*128 + p, n]
        shard = qw8[:, sl].reshape(KC, P, NL).transpose(1, 0, 2)
        in_maps.append(
            {
                "xq": xq,
                "qw": np.ascontiguousarray(shard),
                "ws": np.ascontiguousarray(
                    np.broadcast_to(w_scales[sl][None, :], (P, NL))
                ),
                "bs": np.ascontiguousarray(
                    np.broadcast_to(bias[sl][None, :], (P, NL))
                ),
                "asc": asc,
            }
        )
    return in_maps


def kernel(x, qweight, w_scales, bias):
    global _cached
    if _cached is None:
        _cached = _build_nc()
    nc, _ = _cached

    in_maps = _prep_inputs(x, qweight, w_scales, bias)
    res = None
    err = None
    for _ in range(3):  # retry transient device errors
        try:
            res = bass_utils.run_bass_kernel_spmd(
                nc, in_maps, core_ids=list(range(N_CORES))
            )
            break
        except Exception as e:  # noqa: BLE001
            err = e
    if res is None:
        raise err

    out = np.empty((TOK, N), dtype=np.float32)
    for i in range(N_CORES):
        out[:, i * NL : (i + 1) * NL] = res.results[i]["out"].reshape(TOK, NL)
    return out.reshape(B, S, N)


# revision 26
# speedup vs baseline: 1.1755x; 1.1755x over previous
"""AWQ W4A8 linear (x:[8,32,8192] f32, qweight:[8192,8192] int4-range int32,
w_scales/bias:[8192] f32) -> [8,32,8192] f32 on 8 trn2 NeuronCores.

Column-parallel sharding: qweight / w_scales / bias are split along N
(output channels) across the 8 cores; x — quantized per-token on the host
exactly as the reference does — and the per-token act_scales are
replicated. Each core computes an exact integer GEMM of
x_q [256,8192] @ qw_shard [8192,1024], applies the per-token/per-channel
dequant + bias epilogue, and writes its [256,1024] slice; the host
concatenates the slices.

Numerics: x_q in [-127,127] ships as bf16 and qw in [-8,7] ships as fp8e4
(both exactly representable), and the PE's mixed bf16 x fp8 matmul
accumulates exactly in fp32 PSUM (every product/sum is an integer < 2^24),
so the result matches the reference bit-for-bit while weight HBM traffic
drops 4x vs the int32 input encoding.

The device program is raw Bass (no TileContext) with hand-placed
semaphores. The PE stream is the hard floor (256 matmuls of N=512 at
~213 ns warm = ~55 us), so the design keeps the PE saturated:

- Weights (8 MB) and activations (4 MB) are fully resident in SBUF — no
  slot ring, so DMA runs arbitrarily far ahead with zero backpressure.
- Early weight ranges alternate between the two hardware DGE queues
  (sync + scalar) so the first ~1 MB lands at twice the single-queue
  ramp rate; x rides the gpsimd SWDGE queue (piece 0 goes early on
  scalar), with the 1 MB of scale/bias constants last.
- A short burst of dummy matmuls on garbage SBUF runs during the DGE
  spin-up so the HAM clock-gate reaches 2.4 GHz in dead time. (Dummy
  work must stay minimal: sustained extra PE load trips the P0 power
  downclock, 2.4 -> 2.0 GHz, slowing every real matmul.)
- Per-range completion semaphores: a DMA's +16 lands as 16 per-spray-
  engine increments which interleave across in-flight DMAs, so a single
  counting semaphore is NOT range-ordered.
- Chunks 0..47 interleave all 4 PSUM tiles; chunks 48..63 run
  tile-by-tile so each tile's dequant epilogue (DVE) and output store
  (sync queue) overlap the tail matmuls; the last tile's epilogue is
  further split in halves. Only ~2 us of tail is exposed.
"""

from contextlib import ExitStack

import numpy as np

import concourse.bass as bass
import concourse.mybir as mybir
import concourse.bass_utils as bass_utils
from concourse.dt import dt as cdt

N_CORES = 8
P = 128
B, S, K, N = 8, 32, 8192, 8192
TOK = B * S                      # 256 tokens
NL = N // N_CORES                # 1024 output channels per core
KC = K // P                      # 64 contraction chunks of 128
EPS = 1e-8

# DMA groups: weights stream on the sync queue, x + constants on the
# scalar queue (both hardware DGE; gpsimd's software DGE is too slow for
# bulk traffic). Small leading groups so chunk 0 lands ASAP.
W_SIZES = [1, 1, 2, 4] + [8] * 7
X_SIZES = [1, 1, 2, 4] + [8] * 7
PH1 = 48                         # chunks 0..PH1-1 interleave all 4 tiles
NWARM = 10                       # warm-up dummy matmuls (N=512)

assert sum(W_SIZES) == KC and sum(X_SIZES) == KC

_cached = None


def _ranges(sizes):
    out, acc = [], 0
    for s in sizes:
        out.append((acc, acc + s))
        acc += s
    return out


W_RANGES = _ranges(W_SIZES)
X_RANGES = _ranges(X_SIZES)


def _range_of(c, ranges):
    for i, (a, b) in enumerate(ranges):
        if a <= c < b:
            return i
    raise ValueError(c)


def _build_nc():
    nc = bass.Bass(
        "TRN2",
        target_bir_lowering=False,
        debug=False,
        enable_asserts=False,
        num_devices=N_CORES,
    )
    dt = mybir.dt

    xq_d = nc.dram_tensor("xq", [P, KC, TOK], dt.bfloat16, kind="ExternalInput")
    qw_d = nc.dram_tensor("qw", [P, KC, NL], dt.float8e4, kind="ExternalInput")
    ws_d = nc.dram_tensor("ws", [P, NL], dt.float32, kind="ExternalInput")
    bs_d = nc.dram_tensor("bs", [P, NL], dt.float32, kind="ExternalInput")
    as_d = nc.dram_tensor("asc", [P, 2], dt.float32, kind="ExternalInput")
    out_d = nc.dram_tensor("out", [2, P, NL], dt.float32, kind="ExternalOutput")

    ctx = ExitStack()
    xq_s = ctx.enter_context(nc.sbuf_tensor("xq_s", [P, KC, TOK], dt.bfloat16))
    w_s = ctx.enter_context(nc.sbuf_tensor("w_s", [P, KC, NL], dt.float8e4))
    ws_s = ctx.enter_context(nc.sbuf_tensor("ws_s", [P, NL], dt.float32))
    bs_s = ctx.enter_context(nc.sbuf_tensor("bs_s", [P, NL], dt.float32))
    as_s = ctx.enter_context(nc.sbuf_tensor("as_s", [P, 2], dt.float32))
    t_s = ctx.enter_context(nc.sbuf_tensor("t_s", [P, 4, 512], dt.float32))
    o_s = ctx.enter_context(nc.sbuf_tensor("o_s", [P, 4, 512], dt.float32))
    # never DMA'd: garbage operands for the PE warm-up burst
    dum_s = ctx.enter_context(nc.sbuf_tensor("dum_s", [P, 512], dt.bfloat16))

    ps = [
        ctx.enter_context(nc.psum_tensor(f"ps{i}", [P, 512], dt.float32))
        for i in range(4)  # (m,n): 00,01,10,11
    ]
    ps_warm = ctx.enter_context(nc.psum_tensor("ps_warm", [P, 512], dt.float32))

    sems = {}

    def sem(name):
        sems[name] = ctx.enter_context(nc.semaphore(name))
        return sems[name]

    s_wr = [sem(f"s_wr{g}") for g in range(len(W_RANGES))]
    s_xp = [sem(f"s_xp{i}") for i in range(len(X_RANGES))]
    s_cst = sem("s_cst")  # asc + ws + bs, +16 each (total 48 = all done)
    s_ps = sem("s_ps")    # tile accumulation complete, +1 per tile in order
    s_dve = sem("s_dve")  # DVE same-engine RAW ordering
    s_ep = sem("s_ep")    # epilogue complete, +1 per unit in order
    s_out = sem("s_out")  # output stores (DGE requires sync info)

    # Epilogue/store units: (m, col0, col1) within the core's NL columns.
    # The last tile's epilogue is split in halves so only ~half a tile's
    # epilogue + store is exposed after the final matmul.
    EP_UNITS = [(0, 0, 512), (0, 512, 1024), (1, 0, 512),
                (1, 512, 768), (1, 768, 1024)]
    EP_OSL = [(0, slice(0, 512)), (1, slice(0, 512)), (2, slice(0, 512)),
              (3, slice(0, 256)), (3, slice(256, 512))]
    EP_WAIT = [1, 2, 3, 4, 4]  # s_ps value each unit waits for

    def wdma(eng, r):
        a, b = W_RANGES[r]
        eng.dma_start(w_s[:, a:b, :], qw_d.ap()[:, a:b, :]).then_inc(s_wr[r], 16)

    def xdma(eng, i):
        a, b = X_RANGES[i]
        eng.dma_start(xq_s[:, a:b, :], xq_d.ap()[:, a:b, :]).then_inc(s_xp[i], 16)

    # Issue the critical first DMAs before anything else: the DGE spin-up
    # takes ~2-3us and runs while the program clears semaphores / warms up.
    # Prior executions fully drained (Block exit drains), so clearing the
    # semaphores below without a dma_reset is safe.
    wdma(nc.sync, 0)
    xdma(nc.scalar, 0)

    # Zero our semaphores up front (a previous execution of this NEFF leaves
    # them at their final values), then barrier so no engine runs ahead.
    nums = sorted(s.num for s in sems.values())
    lo = 0
    while lo < len(nums):
        hi = lo
        while hi + 1 < len(nums) and nums[hi + 1] == nums[hi] + 1:
            hi += 1
        nc.gpsimd.sem_clear(range(nums[lo], nums[hi] + 1))
        lo = hi + 1
    nc.all_engine_barrier()

    with nc.Block() as block:

        @block.sync
        def _(sync):
            for r in range(1, len(W_RANGES)):
                wdma(sync, r)
            for idx in range(len(EP_UNITS)):
                m, c0, c1 = EP_UNITS[idx]
                sync.wait_ge(s_ep, idx + 1)
                sync.dma_start(
                    out_d.ap()[m][:, c0:c1], o_s[:, EP_OSL[idx][0], EP_OSL[idx][1]]
                ).then_inc(s_out, 16)

        @block.scalar
        def _(scalar):
            for i in range(1, len(X_RANGES)):
                xdma(scalar, i)
            scalar.dma_start(as_s[:], as_d.ap()).then_inc(s_cst, 16)
            scalar.dma_start(ws_s[:], ws_d.ap()).then_inc(s_cst, 16)
            scalar.dma_start(bs_s[:], bs_d.ap()).then_inc(s_cst, 16)

        @block.tensor
        def _(tensor):
            # Warm-up burst: garbage matmuls into a scratch PSUM bank keep
            # the PE busy through the HAM activity window while the first
            # real chunks stream in.
            for _ in range(NWARM):
                tensor.matmul(
                    ps_warm.ap(), dum_s[:, 0:P], dum_s[:], start=True, stop=True
                )

            cur_wr = -1
            cur_xp = -1

            def need(c):
                nonlocal cur_wr, cur_xp
                r = _range_of(c, W_RANGES)
                if r > cur_wr:
                    tensor.wait_ge(s_wr[r], 16)
                    cur_wr = r
                i = _range_of(c, X_RANGES)
                if i > cur_xp:
                    tensor.wait_ge(s_xp[i], 16)
                    cur_xp = i

            def mm(c, m, n0, n1, stop=False, inc=False):
                inst = tensor.matmul(
                    ps[2 * m + (n0 >= 512)].ap()[:, n0 % 512 : n0 % 512 + (n1 - n0)],
                    xq_s[:, c, P * m : P * (m + 1)],
                    w_s[:, c, n0:n1],
                    start=(c == 0),
                    stop=stop,
                )
                if inc:
                    inst.then_inc(s_ps, 1)

            # Phase 1: chunks 0..PH1-1, all 4 tiles per chunk (follows DMA)
            for c in range(PH1):
                need(c)
                for m in range(2):
                    mm(c, m, 0, 512)
                    mm(c, m, 512, 1024)

            # Phase 2: chunks PH1..KC-1 tile-by-tile; each tile's epilogue
            # and store overlap the next tile's matmuls.
            need(KC - 1)
            for m, n in [(0, 0), (0, 1), (1, 0), (1, 1)]:
                for c in range(PH1, KC):
                    last = c == KC - 1
                    mm(c, m, 512 * n, 512 * (n + 1), stop=last, inc=last)

        @block.vector
        def _(vector):
            vector.wait_ge(s_cst, 48)
            for idx in range(len(EP_UNITS)):
                m, c0, c1 = EP_UNITS[idx]
                osl = EP_OSL[idx]
                bank = 2 * m + (c0 >= 512)
                psl = slice(c0 % 512, c0 % 512 + (c1 - c0))
                vector.wait_ge(s_ps, EP_WAIT[idx])
                vector.scalar_tensor_tensor(
                    t_s[:, osl[0], osl[1]],
                    ps[bank].ap()[:, psl],
                    as_s[:, m : m + 1],
                    ws_s[:, c0:c1],
                    mybir.AluOpType.mult,
                    mybir.AluOpType.mult,
                ).then_inc(s_dve, 1)
                # DVE is deeply pipelined: same-engine RAW needs a sem
                vector.wait_ge(s_dve, idx + 1)
                vector.tensor_add(
                    o_s[:, osl[0], osl[1]], t_s[:, osl[0], osl[1]], bs_s[:, c0:c1]
                ).then_inc(s_ep, 1)

    return nc, ctx


def _prep_inputs(x, qweight, w_scales, bias):
    bf16 = cdt.np(mybir.dt.bfloat16)
    fp8 = cdt.np(mybir.dt.float8e4)

    x2 = np.asarray(x, dtype=np.float32).reshape(TOK, K)
    max_abs = np.max(np.abs(x2), axis=-1, keepdims=True)
    act_scales = np.maximum(max_abs / np.float32(127.0), np.float32(EPS)).astype(
        np.float32
    )
    x_q = np.clip(np.round(x2 / act_scales), -127, 127).astype(np.float32)

    # [TOK, K] -> K-major [P, KC, TOK]: xq[p, c, t] = x_q[t, c*128 + p]
    xq = np.ascontiguousarray(
        x_q.T.reshape(KC, P, TOK).transpose(1, 0, 2).astype(bf16)
    )

    # act_scales arranged per m-tile: asc[p, m] = act_scales[m*128 + p]
    asc = np.ascontiguousarray(act_scales.reshape(2, P).T.astype(np.float32))

    # int4-range weights are exactly representable in fp8 e4m3
    qw8 = np.asarray(qweight, dtype=np.int8).astype(fp8)
    w_scales = np.asarray(w_scales, dtype=np.float32)
    bias = np.asarray(bias, dtype=np.float32)

    in_maps = []
    for i in range(N_CORES):
        sl = slice(i * NL, (i + 1) * NL)
        # [K, NL] -> p-major [P, KC, NL]: qw[p, c, n] = shard[c*128 + p, n]
        shard = qw8[:, sl].reshape(KC, P, NL).transpose(1, 0, 2)
        in_maps.append(
            {
                "xq": xq,
                "qw": np.ascontiguousarray(shard),
                "ws": np.ascontiguousarray(
                    np.broadcast_to(w_scales[sl][None, :], (P, NL))
                ),
                "bs": np.ascontiguousarray(
                    np.broadcast_to(bias[sl][None, :], (P, NL))
                ),
                "asc": asc,
            }
        )
    return in_maps


def kernel(x, qweight, w_scales, bias):
    global _cached
    if _cached is None:
        _cached = _build_nc()
    nc, _ = _cached

    in_maps = _prep_inputs(x, qweight, w_scales, bias)
    res = None
    err = None
    for _ in range(3):  # retry transient device errors
        try:
            res = bass_utils.run_bass_kernel_spmd(
                nc, in_maps, core_ids=list(range(N_CORES))
            )
            break
        except Exception as e:  # noqa: BLE001
            err = e
    if res is None:
        raise err

    out = np.empty((TOK, N), dtype=np.float32)
    for i in range(N_CORES):
        out[:, i * NL : (i + 1) * NL] = res.results[i]["out"].reshape(TOK, NL)
    return out.reshape(B, S, N)
